# revision 14
# baseline (speedup 1.0000x reference)
"""Householder reflection kernel for Trainium2 (Bass/Tile), 8-core SPMD.

Computes z - 2 * v * (v.z)/(v.v) rowwise over [16384, 1024] f32 inputs.
Pure data-parallel: batch dim split evenly across 8 NeuronCores.
"""

import sys

try:
    import concourse  # noqa: F401  (via PYTHONPATH in the normal env)
except ImportError:
    sys.path.append("/opt/trn_rl_repo")

import numpy as np

import concourse.bass as bass
import concourse.tile as tile
from concourse import bacc, mybir
from concourse.alu_op_type import AluOpType
from concourse.bass_utils import run_bass_kernel_spmd

B, L = 16384, 1024
NCORES = 8
SHARD = B // NCORES          # 2048 rows per core
P = 128                      # SBUF partitions
# rows-per-partition per tile; big tiles amortize DMA overhead, the small
# trailing tiles shorten the end-of-kernel load->compute->store drain
TILE_PLAN = [4, 4, 4, 1, 1, 1, 1]
assert P * sum(TILE_PLAN) == SHARD
F32 = mybir.dt.float32


def _build_nc() -> bass.Bass:
    nc = bacc.Bacc("TRN2", target_bir_lowering=False)

    v = nc.declare_dram_parameter("v", [SHARD, L], F32, isOutput=False)
    z = nc.declare_dram_parameter("z", [SHARD, L], F32, isOutput=False)
    o = nc.declare_dram_parameter("o", [SHARD, L], F32, isOutput=True)

    with tile.TileContext(nc) as tc:
        with (
            tc.tile_pool(name="in", bufs=4) as in_pool,
            tc.tile_pool(name="ot", bufs=8) as ot_pool,
            tc.tile_pool(name="scratch", bufs=1) as scratch_pool,
            tc.tile_pool(name="stats", bufs=4) as stats_pool,
        ):
            row = 0
            for R in TILE_PLAN:
                rows = P * R
                # partition p holds rows row + p*R .. row + p*R + R-1,
                # one contiguous DRAM block per tile
                vd = v[row : row + rows, :].rearrange("(p r) d -> p (r d)", p=P, r=R)
                zd = z[row : row + rows, :].rearrange("(p r) d -> p (r d)", p=P, r=R)
                od = o[row : row + rows, :].rearrange("(p r) d -> r p d", p=P, r=R)
                row += rows

                vt = in_pool.tile([P, R * L], F32, tag="v")
                zt = in_pool.tile([P, R * L], F32, tag="z")
                nc.sync.dma_start(out=vt[:], in_=vd)
                nc.sync.dma_start(out=zt[:], in_=zd)

                vt3 = vt.rearrange("p (r d) -> p r d", r=R)
                zt3 = zt.rearrange("p (r d) -> p r d", r=R)

                vv = stats_pool.tile([P, R], F32, tag="vv")
                vz = stats_pool.tile([P, R], F32, tag="vz")
                sq = scratch_pool.tile([P, L], F32, tag="sq")
                prod = scratch_pool.tile([P, L], F32, tag="prod")

                for r in range(R):
                    # ACT engine: vv[:, r] = sum(v*v) over the row
                    nc.scalar.activation(
                        out=sq[:],
                        in_=vt3[:, r, :],
                        func=mybir.ActivationFunctionType.Square,
                        accum_out=vv[:, r : r + 1],
                    )
                    # DVE: vz[:, r] = sum(-2 * v * z); elementwise result is
                    # scratch, only the accumulation matters
                    nc.vector.scalar_tensor_tensor(
                        out=prod[:],
                        in0=vt3[:, r, :],
                        scalar=-2.0,
                        in1=zt3[:, r, :],
                        op0=AluOpType.mult,
                        op1=AluOpType.mult,
                        accum_out=vz[:, r : r + 1],
                    )

                # ratio = (-2 * v.z) / (v.v), per row
                rvv = stats_pool.tile([P, R], F32, tag="rvv")
                ratio = stats_pool.tile([P, R], F32, tag="ratio")
                nc.vector.reciprocal(rvv[:], vv[:])
                nc.vector.tensor_mul(ratio[:], vz[:], rvv[:])

                for r in range(R):
                    ot = ot_pool.tile([P, L], F32, tag="ot")
                    # out = (v * ratio) + z  ==  z - 2 v (v.z)/(v.v)
                    nc.vector.scalar_tensor_tensor(
                        out=ot[:],
                        in0=vt3[:, r, :],
                        scalar=ratio[:, r : r + 1],
                        in1=zt3[:, r, :],
                        op0=AluOpType.mult,
                        op1=AluOpType.add,
                    )
                    # store each 512 KiB segment as soon as it's ready, on
                    # the ACT HWDGE ring so it overlaps in-flight loads
                    nc.scalar.dma_start(out=od[r], in_=ot[:])

    nc.compile()
    return nc


_NC_CACHE = None


def _get_nc() -> bass.Bass:
    global _NC_CACHE
    if _NC_CACHE is None:
        _NC_CACHE = _build_nc()
    return _NC_CACHE


def run_sharded(v: np.ndarray, z: np.ndarray, **spmd_kwargs):
    """Shard inputs over cores, run, return (full_output, BassKernelResults)."""
    nc = _get_nc()
    v = np.ascontiguousarray(v, dtype=np.float32)
    z = np.ascontiguousarray(z, dtype=np.float32)
    in_maps = [
        {
            "v": v[i * SHARD : (i + 1) * SHARD],
            "z": z[i * SHARD : (i + 1) * SHARD],
        }
        for i in range(NCORES)
    ]
    res = run_bass_kernel_spmd(nc, in_maps, list(range(NCORES)), **spmd_kwargs)
    out = np.concatenate([np.asarray(r["o"]) for r in res.results], axis=0)
    return out, res


def kernel(v: np.ndarray, z: np.ndarray) -> np.ndarray:
    out, _ = run_sharded(v, z)
    return out.astype(np.float32)


# revision 17
# speedup vs baseline: 1.0591x; 1.0591x over previous
"""Householder reflection kernel for Trainium2 (Bass/Tile), 8-core SPMD.

Computes z - 2 * v * (v.z)/(v.v) rowwise over [16384, 1024] f32 inputs.
Pure data-parallel: batch dim split evenly across 8 NeuronCores.
"""

import sys

try:
    import concourse  # noqa: F401  (via PYTHONPATH in the normal env)
except ImportError:
    sys.path.append("/opt/trn_rl_repo")

import numpy as np

import concourse.bass as bass
import concourse.tile as tile
from concourse import bacc, mybir
from concourse.alu_op_type import AluOpType
from concourse.bass_utils import run_bass_kernel_spmd

B, L = 16384, 1024
NCORES = 8
SHARD = B // NCORES          # 2048 rows per core
P = 128                      # SBUF partitions
# rows-per-partition per tile; big tiles amortize DMA overhead, the small
# trailing tiles shorten the end-of-kernel load->compute->store drain
TILE_PLAN = [4, 4, 4, 1, 1, 1, 1]
assert P * sum(TILE_PLAN) == SHARD
F32 = mybir.dt.float32


def _build_nc() -> bass.Bass:
    nc = bacc.Bacc("TRN2", target_bir_lowering=False)

    v = nc.declare_dram_parameter("v", [SHARD, L], F32, isOutput=False)
    z = nc.declare_dram_parameter("z", [SHARD, L], F32, isOutput=False)
    o = nc.declare_dram_parameter("o", [SHARD, L], F32, isOutput=True)

    with tile.TileContext(nc) as tc:
        with (
            tc.tile_pool(name="in", bufs=4) as in_pool,
            tc.tile_pool(name="ot", bufs=3) as ot_pool,
            tc.tile_pool(name="scratch", bufs=1) as scratch_pool,
            tc.tile_pool(name="stats", bufs=4) as stats_pool,
        ):
            row = 0
            for R in TILE_PLAN:
                rows = P * R
                # partition p holds rows row + p*R .. row + p*R + R-1,
                # one contiguous DRAM block per tile
                vd = v[row : row + rows, :].rearrange("(p r) d -> p (r d)", p=P, r=R)
                zd = z[row : row + rows, :].rearrange("(p r) d -> p (r d)", p=P, r=R)
                od = o[row : row + rows, :].rearrange("(p r) d -> p (r d)", p=P, r=R)
                row += rows

                vt = in_pool.tile([P, R * L], F32, tag="v")
                zt = in_pool.tile([P, R * L], F32, tag="z")
                nc.sync.dma_start(out=vt[:], in_=vd)
                nc.sync.dma_start(out=zt[:], in_=zd)

                vt3 = vt.rearrange("p (r d) -> p r d", r=R)
                zt3 = zt.rearrange("p (r d) -> p r d", r=R)

                vv = stats_pool.tile([P, R], F32, tag="vv")
                vz = stats_pool.tile([P, R], F32, tag="vz")
                sq = scratch_pool.tile([P, L], F32, tag="sq")
                prod = scratch_pool.tile([P, L], F32, tag="prod")

                for r in range(R):
                    # ACT engine: vv[:, r] = sum(v*v) over the row
                    nc.scalar.activation(
                        out=sq[:],
                        in_=vt3[:, r, :],
                        func=mybir.ActivationFunctionType.Square,
                        accum_out=vv[:, r : r + 1],
                    )
                    # DVE: vz[:, r] = sum(-2 * v * z); elementwise result is
                    # scratch, only the accumulation matters
                    nc.vector.scalar_tensor_tensor(
                        out=prod[:],
                        in0=vt3[:, r, :],
                        scalar=-2.0,
                        in1=zt3[:, r, :],
                        op0=AluOpType.mult,
                        op1=AluOpType.mult,
                        accum_out=vz[:, r : r + 1],
                    )

                # ratio = (-2 * v.z) / (v.v), per row
                rvv = stats_pool.tile([P, R], F32, tag="rvv")
                ratio = stats_pool.tile([P, R], F32, tag="ratio")
                nc.vector.reciprocal(rvv[:], vv[:])
                nc.vector.tensor_mul(ratio[:], vz[:], rvv[:])

                ot = ot_pool.tile([P, R * L], F32, tag="ot")
                ot3 = ot.rearrange("p (r d) -> p r d", r=R)
                for r in range(R):
                    # out = (v * ratio) + z  ==  z - 2 v (v.z)/(v.v)
                    nc.vector.scalar_tensor_tensor(
                        out=ot3[:, r, :],
                        in0=vt3[:, r, :],
                        scalar=ratio[:, r : r + 1],
                        in1=zt3[:, r, :],
                        op0=AluOpType.mult,
                        op1=AluOpType.add,
                    )
                # one contiguous store per tile, on the ACT HWDGE ring so it
                # overlaps in-flight loads on the sync ring
                nc.scalar.dma_start(out=od, in_=ot[:])

    nc.compile()
    return nc


_NC_CACHE = None


def _get_nc() -> bass.Bass:
    global _NC_CACHE
    if _NC_CACHE is None:
        _NC_CACHE = _build_nc()
    return _NC_CACHE


def run_sharded(v: np.ndarray, z: np.ndarray, **spmd_kwargs):
    """Shard inputs over cores, run, return (full_output, BassKernelResults)."""
    nc = _get_nc()
    v = np.ascontiguousarray(v, dtype=np.float32)
    z = np.ascontiguousarray(z, dtype=np.float32)
    in_maps = [
        {
            "v": v[i * SHARD : (i + 1) * SHARD],
            "z": z[i * SHARD : (i + 1) * SHARD],
        }
        for i in range(NCORES)
    ]
    res = run_bass_kernel_spmd(nc, in_maps, list(range(NCORES)), **spmd_kwargs)
    out = np.concatenate([np.asarray(r["o"]) for r in res.results], axis=0)
    return out, res


def kernel(v: np.ndarray, z: np.ndarray) -> np.ndarray:
    out, _ = run_sharded(v, z)
    return out.astype(np.float32)


# revision 21
# speedup vs baseline: 1.1930x; 1.1264x over previous
"""Householder reflection kernel for Trainium2 (Bass/Tile), 8-core SPMD.

Computes z - 2 * v * (v.z)/(v.v) rowwise over [16384, 1024] f32 inputs.
Pure data-parallel: batch dim split evenly across 8 NeuronCores.
"""

import sys

try:
    import concourse  # noqa: F401  (via PYTHONPATH in the normal env)
except ImportError:
    sys.path.append("/opt/trn_rl_repo")

import numpy as np

import concourse.bass as bass
import concourse.tile as tile
from concourse import bacc, mybir
from concourse.alu_op_type import AluOpType
from concourse.bass_utils import run_bass_kernel_spmd

B, L = 16384, 1024
NCORES = 8
SHARD = B // NCORES          # 2048 rows per core
P = 128                      # SBUF partitions
# rows-per-partition per tile; big tiles amortize DMA overhead, the small
# trailing tiles shorten the end-of-kernel load->compute->store drain
TILE_PLAN = [4, 4, 4, 2, 2]
assert P * sum(TILE_PLAN) == SHARD
F32 = mybir.dt.float32


def _build_nc() -> bass.Bass:
    nc = bacc.Bacc("TRN2", target_bir_lowering=False)

    v = nc.declare_dram_parameter("v", [SHARD, L], F32, isOutput=False)
    z = nc.declare_dram_parameter("z", [SHARD, L], F32, isOutput=False)
    o = nc.declare_dram_parameter("o", [SHARD, L], F32, isOutput=True)

    with tile.TileContext(nc) as tc:
        with (
            tc.tile_pool(name="in", bufs=4) as in_pool,
            tc.tile_pool(name="ot", bufs=3) as ot_pool,
            tc.tile_pool(name="scratch", bufs=1) as scratch_pool,
            tc.tile_pool(name="stats", bufs=4) as stats_pool,
        ):
            row = 0
            for R in TILE_PLAN:
                rows = P * R
                # partition p holds rows row + p*R .. row + p*R + R-1,
                # one contiguous DRAM block per tile
                vd = v[row : row + rows, :].rearrange("(p r) d -> p (r d)", p=P, r=R)
                zd = z[row : row + rows, :].rearrange("(p r) d -> p (r d)", p=P, r=R)
                od = o[row : row + rows, :].rearrange("(p r) d -> p (r d)", p=P, r=R)
                row += rows

                vt = in_pool.tile([P, R * L], F32, tag="v")
                zt = in_pool.tile([P, R * L], F32, tag="z")
                nc.sync.dma_start(out=vt[:], in_=vd)
                nc.sync.dma_start(out=zt[:], in_=zd)

                vt3 = vt.rearrange("p (r d) -> p r d", r=R)
                zt3 = zt.rearrange("p (r d) -> p r d", r=R)

                vv = stats_pool.tile([P, R], F32, tag="vv")
                vz = stats_pool.tile([P, R], F32, tag="vz")
                sq = scratch_pool.tile([P, L], F32, tag="sq")
                prod = scratch_pool.tile([P, L], F32, tag="prod")

                for r in range(R):
                    # ACT engine: vv[:, r] = sum(v*v) over the row
                    nc.scalar.activation(
                        out=sq[:],
                        in_=vt3[:, r, :],
                        func=mybir.ActivationFunctionType.Square,
                        accum_out=vv[:, r : r + 1],
                    )
                    # DVE: vz[:, r] = sum(-2 * v * z); elementwise result is
                    # scratch, only the accumulation matters
                    nc.vector.scalar_tensor_tensor(
                        out=prod[:],
                        in0=vt3[:, r, :],
                        scalar=-2.0,
                        in1=zt3[:, r, :],
                        op0=AluOpType.mult,
                        op1=AluOpType.mult,
                        accum_out=vz[:, r : r + 1],
                    )

                # ratio = (-2 * v.z) / (v.v), per row
                rvv = stats_pool.tile([P, R], F32, tag="rvv")
                ratio = stats_pool.tile([P, R], F32, tag="ratio")
                nc.vector.reciprocal(rvv[:], vv[:])
                nc.vector.tensor_mul(ratio[:], vz[:], rvv[:])

                ot = ot_pool.tile([P, R * L], F32, tag="ot")
                ot3 = ot.rearrange("p (r d) -> p r d", r=R)
                for r in range(R):
                    # out = (v * ratio) + z  ==  z - 2 v (v.z)/(v.v)
                    nc.vector.scalar_tensor_tensor(
                        out=ot3[:, r, :],
                        in0=vt3[:, r, :],
                        scalar=ratio[:, r : r + 1],
                        in1=zt3[:, r, :],
                        op0=AluOpType.mult,
                        op1=AluOpType.add,
                    )
                # one contiguous store per tile, on the ACT HWDGE ring so it
                # overlaps in-flight loads on the sync ring
                nc.scalar.dma_start(out=od, in_=ot[:])

    nc.compile()
    return nc


_NC_CACHE = None


def _get_nc() -> bass.Bass:
    global _NC_CACHE
    if _NC_CACHE is None:
        _NC_CACHE = _build_nc()
    return _NC_CACHE


def run_sharded(v: np.ndarray, z: np.ndarray, **spmd_kwargs):
    """Shard inputs over cores, run, return (full_output, BassKernelResults)."""
    nc = _get_nc()
    v = np.ascontiguousarray(v, dtype=np.float32)
    z = np.ascontiguousarray(z, dtype=np.float32)
    in_maps = [
        {
            "v": v[i * SHARD : (i + 1) * SHARD],
            "z": z[i * SHARD : (i + 1) * SHARD],
        }
        for i in range(NCORES)
    ]
    res = run_bass_kernel_spmd(nc, in_maps, list(range(NCORES)), **spmd_kwargs)
    out = np.concatenate([np.asarray(r["o"]) for r in res.results], axis=0)
    return out, res


def kernel(v: np.ndarray, z: np.ndarray) -> np.ndarray:
    out, _ = run_sharded(v, z)
    return out.astype(np.float32)
